# revision 2
# baseline (speedup 1.0000x reference)
"""N-pairs custom loss on 8 Trainium2 NeuronCores.

Math
----
reference computes, with a' = anchor + 1e-6:
    sq[i,j] = ||a'_i||^2 + ||p_j||^2 - 2 a'_i . p_j
    dist    = sqrt(max(sq, 1e-12))
    hinge   = relu(diag(dist)[i] + 1 - dist[i,j])
    loss    = sum over {i : label_i == 1, j != i} hinge / count

Device-side restructuring:
  * Only rows with label==1 contribute -> host compacts those rows
    (K ~ N/2). 2x4 (rows x cols) sharding: each core gets a 2048-row x
    2048-col block of the K x N pair grid, so per-core DMA is
    2 x 270KB = 540KB (vs 1.15MB for pure row sharding) and the DMA
    warm-up no longer gates the pipeline.
  * The PE emits y = (c_i^2 - sq_ij) / (2 c_i) directly (c_i =
    pos_dist_i + MARGIN): the augmented anchor rows are affinely
    transformed on the host, ahat''_i = (c_i^2 e_const - ahat_i)/(2c_i)
    against phat_j = [sqrt(2) p_j, 1, ||p_j||^2] (66 contraction dims).
    Then u = dist/c = sqrt(1 - 2y/c), so the ACT pass is
    sqrt(y * (-2/c_i) + 1) with a per-partition scale and unit bias.
  * Per tile [128 rows x 2048 cols]: 4 matmuls (fp16, fp32 PSUM) ->
    ACT sqrt PSUM->SBUF bf16 -> DVE tensor_scalar (u min 1.0, add)
    whose accumulator gives sum_j min(u,1) per row in the same pass.
    One tile per core instead runs both reduce passes on ACT
    (min(u,1) = u - relu(u-1): sqrt with accum_out=sum u, then a
    relu(u-1) pass with accum): the DVE reduce cadence (~2330ns) is
    slightly above ACT's (~2060ns), so one ACT-offloaded tile levels
    the two queues at ~35us each.
  * Host reduces the small per-core [128, slots] partials:
    total = sum_i c_i*(N - sum_j min(u_ij,1)) - K; loss = total/count.
    The diagonal j==i contributes exactly MARGIN per row.

This walrus build accepts only ONE sync wait per instruction; a
post-serialization pass splits excess waits into EventSemaphore
instructions and fuses Ldweights into self-loading matmuls so walrus's
LDW optimization can pipeline weight loads (see _legalize_bir).
"""

import numpy as np

import concourse.bass as bass
import concourse.mybir as mybir
from concourse import tile
from concourse.bass_utils import run_bass_kernel_spmd

N_CORES = 8
NTOT = 8192               # number of positive embeddings (full N)
D = 64
KAUG = D + 2              # augmented contraction dim
ROW_TILE = 128
ROW_GROUPS = 2
COL_GROUPS = 4
R_PER_CORE = 2048                         # rows per core
C_PER_CORE = NTOT // COL_GROUPS           # 2048 cols per core
N_ROW_TILES = R_PER_CORE // ROW_TILE      # 16 tiles per core
ROW_CAP = ROW_GROUPS * R_PER_CORE         # 4096 label-1 rows per launch
HOST_TAIL_MAX = 256   # rows beyond full launches handled on host (numpy)
COL_CHUNK = 2048                          # PSUM tile width (4 banks)
MM_FREE = 512                             # moving free dim per matmul
MARGIN = 1.0
EPS = 1e-6
N_ACT_TILES = 1          # tiles whose reduce runs fully on ACT
ACT_TILES = (N_ROW_TILES - 2,)   # placed late: shortens the DVE drain

_CACHED_NC = None
last_results = None       # BassKernelResults of the most recent launch
TRACE = False             # set True (e.g. from test.py) to capture a profile
TRACE_CORES = None        # e.g. list(range(8)) to profile all cores


def _build_nc():
    nc = bass.Bass()
    ahat = nc.dram_tensor("ahat", [KAUG, R_PER_CORE], mybir.dt.float16,
                          kind="ExternalInput")
    phat = nc.dram_tensor("phat", [KAUG, C_PER_CORE], mybir.dt.float16,
                          kind="ExternalInput")
    svec = nc.dram_tensor("svec", [ROW_TILE, N_ROW_TILES], mybir.dt.float32,
                          kind="ExternalInput")
    # accumulator columns: one sum_j min(u,1) slot per tile, +2 for the
    # head/tail split halves, +1 extra for each ACT-offloaded tile's
    # sum(relu(u-1)) term (its base slot holds sum u)
    n_acc = N_ROW_TILES + 2 + N_ACT_TILES
    acc_out = nc.dram_tensor("acc", [ROW_TILE, n_acc],
                             mybir.dt.float32, kind="ExternalOutput")

    with tile.TileContext(nc) as tc:
        with (
            tc.tile_pool(name="const", bufs=1) as const_pool,
            # one SBUF slot per tile's u: slot reuse would force extra
            # ACT/DVE WAR semaphore waits (each costing a split EVSEM)
            tc.tile_pool(name="upool", bufs=N_ROW_TILES + 2) as u_pool,
            tc.tile_pool(name="psum", bufs=2, space="PSUM") as psum_pool,
        ):
            ahat_sb = const_pool.tile([KAUG, R_PER_CORE], mybir.dt.float16)
            phat_sb = const_pool.tile([KAUG, C_PER_CORE], mybir.dt.float16)
            s_sb = const_pool.tile([ROW_TILE, N_ROW_TILES], mybir.dt.float32)
            negone_sb = const_pool.tile([ROW_TILE, 1], mybir.dt.float32)
            junk_sb = const_pool.tile([ROW_TILE, COL_CHUNK], mybir.dt.bfloat16)
            acc_sb = const_pool.tile([ROW_TILE, n_acc], mybir.dt.float32)
            nc.vector.memset(negone_sb[:], -1.0)

            # preload the sqrt activation-table set (~2.7us) as early as
            # possible on the ACT engine: scale=0.0 means the input value
            # is never consumed, so the table load overlaps the NEFF
            # preamble / DMA window instead of gating the first real sqrt
            warm_out = const_pool.tile([1, 1], mybir.dt.float16)
            nc.scalar.activation(warm_out[:], negone_sb[:1, :1],
                                 mybir.ActivationFunctionType.Sqrt,
                                 scale=0.0)

            # DMAs on one queue execute serially; interleave ahat/phat
            # pieces across the sync (HWDGE) and gpsimd (SWDGE) queues in
            # compute order so tile 0 can start as soon as its weights and
            # first 512 columns land.
            nc.gpsimd.dma_start(ahat_sb[:, 0:ROW_TILE], ahat[:, 0:ROW_TILE])
            nc.sync.dma_start(phat_sb[:, 0:512], phat[:, 0:512])
            nc.gpsimd.dma_start(s_sb[:], svec[:])
            nc.sync.dma_start(phat_sb[:, 512:COL_CHUNK],
                              phat[:, 512:COL_CHUNK])
            half = (R_PER_CORE - ROW_TILE) // 2 + ROW_TILE
            nc.gpsimd.dma_start(ahat_sb[:, ROW_TILE:half],
                                ahat[:, ROW_TILE:half])
            nc.gpsimd.dma_start(ahat_sb[:, half:R_PER_CORE],
                                ahat[:, half:R_PER_CORE])

            def emit_tile(r, col0, clen, slot, on_act=False):
                ps = psum_pool.tile([ROW_TILE, COL_CHUNK], mybir.dt.float32,
                                    tag="ps")
                for k in range(clen // MM_FREE):
                    nc.tensor.matmul(
                        ps[:, k * MM_FREE:(k + 1) * MM_FREE],
                        ahat_sb[:, r * ROW_TILE:(r + 1) * ROW_TILE],
                        phat_sb[:, col0 + k * MM_FREE:col0 + (k + 1) * MM_FREE],
                        start=True, stop=True,
                    )
                u_t = u_pool.tile([ROW_TILE, COL_CHUNK],
                                  mybir.dt.bfloat16, tag="u")
                # u = sqrt(y * (-2/c_i) + 1) = dist/c; for the ACT-offloaded
                # tile also accumulate sum(u) in the same pass
                nc.scalar.activation(u_t[:, :clen], ps[:, :clen],
                                     mybir.ActivationFunctionType.Sqrt,
                                     scale=s_sb[:, r:r + 1], bias=1.0,
                                     accum_out=(acc_sb[:, slot:slot + 1]
                                                if on_act else None))
                if on_act:
                    # min(u,1) = u - relu(u-1) exactly: this tile's reduce
                    # runs as a second accumulating ACT pass, freeing the
                    # DVE (whose reduce cadence is the slower of the two).
                    nc.scalar.activation(junk_sb[:, :clen], u_t[:, :clen],
                                         mybir.ActivationFunctionType.Relu,
                                         bias=negone_sb[:, 0:1],
                                         accum_out=acc_sb[:, n_acc - 1:n_acc])
                else:
                    # accum_out[p] = sum_j min(u, 1): op0 computes the out
                    # elements, op1 is the reduce op of the accumulator
                    nc.vector.tensor_scalar(
                        out=junk_sb[:, :clen],
                        in0=u_t[:, :clen],
                        scalar1=1.0, scalar2=0.0,
                        op0=mybir.AluOpType.min,
                        op1=mybir.AluOpType.add,
                        accum_out=acc_sb[:, slot:slot + 1],
                    )

            nc.vector.memset(acc_sb[:], 0.0)
            for r in range(N_ROW_TILES):
                first = (r == 0)
                last = (r == N_ROW_TILES - 1)
                if first:
                    # 512-wide head piece: starts after one DMA piece
                    emit_tile(r, 0, 512, r)
                    emit_tile(r, 512, COL_CHUNK - 512, N_ROW_TILES)
                elif last:
                    # 512-wide final piece: shorter end-of-pipeline drain
                    h = COL_CHUNK - 512
                    emit_tile(r, 0, h, r)
                    emit_tile(r, h, 512, N_ROW_TILES + 1)
                else:
                    emit_tile(r, 0, COL_CHUNK, r, on_act=(r in ACT_TILES))
            nc.sync.dma_start(acc_out[:], acc_sb[:])
    return nc


def _legalize_bir(bir_bytes):
    """Two fixups on the serialized BIR before walrus:

    1. Fuse each standalone Ldweights into its paired (self-loading)
       Matmult: walrus's LDW optimization (background weight buffer ->
       weight loads overlap in-flight matmuls) rejects standalone
       InstLdweights, and without it every LDW/MM pair serializes at the
       full matmul drain latency (~630ns instead of ~430ns per matmul).

    2. This walrus build accepts only ONE sync wait per instruction (two
       on EventSemaphore); Tile emits more on some (epilogue drain, ...).
       Split excess waits into standalone EventSemaphore wait instructions
       on the same engine, inserted immediately before (semantically
       identical: the engine blocks on the same condition set, in order).
    """
    import json as _json
    m = _json.loads(bir_bytes)
    for fn in m["functions"]:
        for blk in fn["blocks"]:
            out = []
            pending_ld = None
            for ins in blk["instructions"]:
                op = ins.get("opcode")
                if op == "Ldweights":
                    if pending_ld is not None:
                        out.append(pending_ld)
                    pending_ld = ins
                    continue
                if op == "Matmult" and pending_ld is not None:
                    if pending_ld["ins"][0] == ins["ins"][1]:
                        ins["ldweights"] = True
                        lsi = pending_ld.get("sync_info") or {}
                        msi = ins.setdefault("sync_info", {})
                        msi["on_wait"] = list(lsi.get("on_wait") or []) + \
                            list(msi.get("on_wait") or [])
                        msi["on_update"] = list(msi.get("on_update") or []) + \
                            list(lsi.get("on_update") or [])
                        pending_ld = None
                    else:
                        out.append(pending_ld)
                        pending_ld = None
                out.append(ins)
            if pending_ld is not None:
                out.append(pending_ld)
            blk["instructions"] = out

    ctr = 0
    for fn in m["functions"]:
        for blk in fn["blocks"]:
            out = []
            for ins in blk["instructions"]:
                si = ins.get("sync_info") or {}
                waits = list(si.get("on_wait") or [])
                cap = 2 if ins.get("opcode") == "EventSemaphore" else 1
                while len(waits) > cap:
                    take, waits = waits[:2], waits[2:]
                    ctr += 1
                    out.append({
                        "engine": ins["engine"],
                        "ins": [], "outs": [],
                        "name": f"waitsplit-{ctr}",
                        "opcode": "EventSemaphore",
                        "sync_info": {"on_update": [], "on_wait": take},
                    })
                if si:
                    si["on_wait"] = waits
                out.append(ins)
            blk["instructions"] = out
    return _json.dumps(m).encode()


def _patch_walrus_flags():
    """Run walrus with --enable-ldw-opt=true (requires self-loading
    matmuls, see _legalize_bir) so weight loads target the background
    weight buffer and overlap in-flight matmuls."""
    import concourse.bass_utils as _bu
    if getattr(_bu.run_command, "_ldwopt_patched", False):
        return
    _orig = _bu.run_command

    def _patched(cmd, **kw):
        if isinstance(cmd, list):
            cmd = ['--enable-ldw-opt=true' if c == '--enable-ldw-opt=false'
                   else c for c in cmd]
        return _orig(cmd, **kw)

    _patched._ldwopt_patched = True
    _bu.run_command = _patched


def _get_nc():
    global _CACHED_NC
    if _CACHED_NC is None:
        _patch_walrus_flags()
        nc = _build_nc()
        orig = nc.to_json_bytes
        nc.to_json_bytes = lambda: _legalize_bir(orig())
        _CACHED_NC = nc
    return _CACHED_NC


def kernel(anchor_embeddings, positive_embeddings, labels):
    global last_results
    a = np.asarray(anchor_embeddings, dtype=np.float32)
    p = np.asarray(positive_embeddings, dtype=np.float32)
    l = np.asarray(labels)
    N = a.shape[0]
    assert N == NTOT and a.shape[1] == D

    idx = np.flatnonzero(l == 1)
    K = int(idx.size)
    count = K * (N - 1)
    if K == 0:
        return np.asarray(0.0, dtype=np.float32)

    # host-side O(N*D) prep: norms, per-row scales, augmentation
    ae = a + np.float32(EPS)
    ae64 = ae.astype(np.float64)
    p64 = p.astype(np.float64)
    a2 = (ae64 * ae64).sum(1)
    p2 = (p64 * p64).sum(1)
    pos_sq = a2 + p2 - 2.0 * (ae64 * p64).sum(1)
    c_all = np.sqrt(np.maximum(pos_sq, 1e-12)) + MARGIN          # f64 [N]

    s2 = np.float64(np.sqrt(2.0))
    phatT = np.empty((KAUG, NTOT), dtype=np.float16)
    phatT[:D] = (s2 * p64).T.astype(np.float16)
    phatT[D] = np.float16(1.0)
    phatT[D + 1] = p2.astype(np.float16)
    phat_slices = [np.ascontiguousarray(phatT[:, g * C_PER_CORE:
                                              (g + 1) * C_PER_CORE])
                   for g in range(COL_GROUPS)]

    nc = _get_nc()
    total = 0.0
    # device launches cover row chunks; a small remainder (< HOST_TAIL_MAX)
    # is cheaper on the host than another full kernel launch
    chunks = []
    pos = 0
    while K - pos > HOST_TAIL_MAX:
        take = min(ROW_CAP, K - pos)
        chunks.append(idx[pos:pos + take])
        pos += take
    tail_rows = idx[pos:]

    for rows in chunks:
        nrows = rows.size
        # ahat'' rows: (c^2 e_const - ahat) / (2c) so the PE emits
        # y = (c^2 - sq)/(2c) directly; padded rows are zero -> y = 0
        # -> u = sqrt(1) = 1 (ignored by the host reduction anyway)
        ahat_rows = np.zeros((ROW_CAP, KAUG), dtype=np.float16)
        cr = c_all[rows][:, None]
        ahat_rows[:nrows, :D] = (s2 * ae64[rows] / (2.0 * cr)).astype(np.float16)
        ahat_rows[:nrows, D] = ((cr * cr - a2[rows][:, None]) /
                                (2.0 * cr)).ravel().astype(np.float16)
        ahat_rows[:nrows, D + 1] = (-1.0 / (2.0 * cr)).ravel().astype(np.float16)
        # per-row ACT scale -2/c (f32); padded rows get -2 (y=0 there)
        s_pad = np.full(ROW_CAP, -2.0, dtype=np.float32)
        s_pad[:nrows] = (-2.0 / c_all[rows]).astype(np.float32)

        in_maps = []
        for core in range(N_CORES):
            rg, cg = divmod(core, COL_GROUPS)
            sl = slice(rg * R_PER_CORE, (rg + 1) * R_PER_CORE)
            in_maps.append({
                "ahat": np.ascontiguousarray(ahat_rows[sl].T),
                "phat": phat_slices[cg],
                "svec": np.ascontiguousarray(
                    s_pad[sl].reshape(N_ROW_TILES, ROW_TILE).T),
            })

        res = run_bass_kernel_spmd(nc, in_maps, core_ids=list(range(N_CORES)),
                                   trace=TRACE, trace_cores=TRACE_CORES)
        last_results = res

        # per-core partials -> per-row sum_j min(u_ij, 1) over that
        # core's 2048 columns; the 4 col-group cores of a row group sum
        msum = np.zeros((ROW_GROUPS, R_PER_CORE), dtype=np.float64)
        n_acc = N_ROW_TILES + 2 + N_ACT_TILES
        for core in range(N_CORES):
            rg, cg = divmod(core, COL_GROUPS)
            acc = res.results[core]["acc"].astype(np.float64)   # [128, n_acc]
            acc[:, 0] += acc[:, N_ROW_TILES]          # head tile, 2nd half
            acc[:, N_ROW_TILES - 1] += acc[:, N_ROW_TILES + 1]  # tail tile
            # ACT-offloaded tiles: sum min(u,1) = sum u - sum relu(u-1)
            for j, t in enumerate(ACT_TILES):
                acc[:, t] -= acc[:, n_acc - N_ACT_TILES + j]
            # acc[:, t] is the per-partition sum for row-tile t:
            # row index within group = t*128 + partition
            msum[rg] += acc[:, :N_ROW_TILES].T.reshape(-1)
        msum = msum.reshape(-1)            # [ROW_CAP] sum_j min(u_ij, 1)
        nreal = nrows
        # sum_j relu(c_i - d_ij) = c_i * (N - sum_j min(u_ij, 1))
        total += (c_all[rows] * (N - msum[:nreal])).sum()

    if tail_rows.size:
        sq_t = (a2[tail_rows][:, None] + p2[None, :]
                - 2.0 * (ae64[tail_rows] @ p64.T))
        d_t = np.sqrt(np.maximum(sq_t, 1e-12))
        total += np.maximum(c_all[tail_rows][:, None] - d_t, 0.0).sum()

    total -= K  # diagonal j==i contributes exactly MARGIN per label-1 row

    loss = total / count
    return np.asarray(loss, dtype=np.float32)
